# revision 21
# baseline (speedup 1.0000x reference)
"""Causal self-attention (B=4, S=2048, D=2048, H=16) on 8 Trainium2 cores.

Sharding: core c -> (batch b = c//2, head-half = c%2, i.e. 8 of 16 heads).
Megatron-style: Wq/Wk/Wv column-parallel (8 heads' rows), Wo row-parallel
(matching 1024 columns).  Each core emits a partial (S, D) output for its
batch; host sums the two half partials per batch and adds bo.

Numerics: bf16 operands with fp32 PSUM accumulation everywhere.
End-to-end L2 rel err ~6e-3.

Device pipeline per core (S=2048, DK=128, 8 local heads):
  Stage 1 (two 4-head passes): QKV projections (fp32r matmuls),
    Q^T,K^T per head in [dk, s] bf16 -> DRAM spill; V in [s, dv] bf16.
  Stage 2, qc-outer / head-inner (all 8 heads' K/V resident in SBUF):
    unit (qc, h):
      S^T pair tiles [k, q] = K^T_chunk.T @ Q^T  (2 chunks -> one PSUM
      pair tile, causally narrowed moving operand)
      es = Exp(S^T * scale) paired activation -> bf16
      causal mask: gpsimd affine_select on diagonal chunks (fill 0)
      denominator: DVE+GpSimd accumulate es chunks -> acc/accg (bf16);
      all-ones matmul partition-reduces+broadcasts -> den (PSUM f32);
      DVE fast reciprocal; ctxn = ctxp(PSUM) * rcp -> bf16 -> DRAM
      ctx^T [dv, q] = sum_k V_chunk.T @ es_chunk (PSUM, narrowed)
    out-projection per qc interleaved into the next qc's units:
      out[q-sub, e] = sum_h ctxs_h.T @ Wo_h  (bf16, PSUM over heads)
"""

import math

import numpy as np

import concourse.bass as bass
import concourse.mybir as mybir
from concourse.bass_utils import run_bass_kernel_spmd
from concourse.tile import TileContext

B, S, D, H = 4, 2048, 2048, 16
DK = 128
NCORES = 8
HPC = H // 2          # 8 heads per core
MLOC = HPC * DK       # 1024 local head dims

F32 = mybir.dt.float32
F32R = mybir.dt.float32r
BF16 = mybir.dt.bfloat16
AF = mybir.ActivationFunctionType


def split_excess_waits(nc, max_waits=1):
    """walrus in this container accepts at most one sem-wait per instruction;
    move excess waits onto wait-only EventSemaphore insts inserted before."""
    ctr = 0
    for f in nc.m.functions:
        for bb in f.blocks:
            new = []
            changed = False
            for inst in bb.instructions:
                si = inst.sync_info
                if si is not None and si.on_wait and len(si.on_wait) > max_waits:
                    changed = True
                    waits = list(si.on_wait)
                    for w in waits[:-max_waits]:
                        ctr += 1
                        ev = mybir.InstEventSemaphore(
                            name=f"waitsplit-{ctr}", ins=[], outs=[],
                            sync_info=mybir.SyncInfo(on_wait=[w], on_update=[]))
                        ev.engine = inst.engine
                        new.append(ev)
                    si.on_wait = waits[-max_waits:]
                new.append(inst)
            if changed:
                bb.instructions = new
    return ctr


def build_nc(seq=S):
    """One core's program: full attention for 1 batch x 8 heads."""
    assert seq % 512 == 0
    NSC = seq // 512          # 512-wide q chunks
    NKC = seq // 128          # 128-wide k chunks
    SCALE = 1.0 / math.sqrt(DK)

    nc = bass.Bass()
    xt = nc.declare_dram_parameter("xt", [D, seq], BF16, isOutput=False)
    wqt = nc.declare_dram_parameter("wqt", [D, MLOC], BF16, isOutput=False)
    wkt = nc.declare_dram_parameter("wkt", [D, MLOC], BF16, isOutput=False)
    wvt = nc.declare_dram_parameter("wvt", [D, MLOC], BF16, isOutput=False)
    wot = nc.declare_dram_parameter("wot", [MLOC, D], BF16, isOutput=False)
    bqt = nc.declare_dram_parameter("bqt", [DK, HPC], F32, isOutput=False)
    bkt = nc.declare_dram_parameter("bkt", [DK, HPC], F32, isOutput=False)
    bvv = nc.declare_dram_parameter("bvv", [MLOC], F32, isOutput=False)
    out = nc.declare_dram_parameter("out", [seq, D], F32, isOutput=True)

    xt_r = xt.rearrange("(dc p) s -> p dc s", p=128)      # [128, 16, seq]
    wqt_r = wqt.rearrange("(dc p) m -> p dc m", p=128)    # [128, 16, 1024]
    wkt_r = wkt.rearrange("(dc p) m -> p dc m", p=128)
    wvt_r = wvt.rearrange("(dc p) m -> p dc m", p=128)
    wot_r = wot.rearrange("(hc p) e -> p hc e", p=128)    # [128, 8, 2048]

    with TileContext(nc) as tc:
        with tc.tile_pool(name="dram", bufs=1, space="DRAM") as dpool, \
             tc.tile_pool(name="kve", bufs=1) as kvE, \
             tc.tile_pool(name="const", bufs=1) as cpool:
            # per-head spill tensors -> fine-grained stage1->stage2 deps
            qd = [dpool.tile([128, seq], BF16, name=f"qd{h}") for h in range(HPC)]
            kd = [dpool.tile([128, seq], BF16, name=f"kd{h}") for h in range(HPC)]
            vd = [dpool.tile([128, NKC, 128], BF16, name=f"vd{h}") for h in range(HPC)]
            k2h, v2h, q3sb = {}, {}, {}

            # ---------------- Stage 1: QKV projections ----------------
            with tc.tile_pool(name="s1w", bufs=2) as wpool, \
                 tc.tile_pool(name="s1wq", bufs=8) as wqpool, \
                 tc.tile_pool(name="s1x", bufs=8) as xpool, \
                 tc.tile_pool(name="s1s", bufs=4) as spool, \
                 tc.tile_pool(name="s1qk", bufs=4, space="PSUM") as qkp, \
                 tc.tile_pool(name="s1v", bufs=2, space="PSUM") as vps:
                bq_sb = cpool.tile([DK, HPC], F32)
                bk_sb = cpool.tile([DK, HPC], F32)
                bv_sb = cpool.tile([128, 2, 512], F32)
                ones_bf = cpool.tile([128, 128], BF16)
                for p_ in range(2):           # head-half pass: heads 4p..4p+3
                    if p_ == 1:
                        # pass-0 heads' K/V/Q(qc=0) loads: allocated above the
                        # still-open stage-1 pools, fire as soon as the pass-0
                        # spills land -> no stage-1 -> stage-2 transition stall
                        for h in range(4):
                            k2h[h] = kvE.tile([128, seq], BF16, tag=f"k2_{h}", name=f"k2_{h}")
                            nc.sync.dma_start(out=k2h[h][:], in_=kd[h][:])
                            v2h[h] = kvE.tile([128, NKC, 128], BF16,
                                              tag=f"v2_{h}", name=f"v2_{h}")
                            nc.sync.dma_start(out=v2h[h][:], in_=vd[h][:])
                            q3 = kvE.tile([128, 512], BF16, tag=f"q3e_{h}", name=f"q3e_{h}")
                            nc.sync.dma_start(out=q3[:], in_=qd[h][:, 0:512])
                            q3sb[(0, h)] = q3
                    # quarter-granularity wq/x loads, interleaved: the first
                    # Q matmul starts after ~1 MB instead of ~6 MB
                    wqq, x0q = [], []
                    for qtr in range(4):
                        w4 = wqpool.tile([128, 4, 512], BF16, tag="wq",
                                        name=f"wq{qtr}")
                        nc.sync.dma_start(
                            out=w4[:],
                            in_=wqt_r[:, 4*qtr:4*qtr+4, p_*512:(p_+1)*512])
                        wqq.append(w4)
                        x4 = xpool.tile([128, 4, 512], BF16, tag="xin",
                                        name=f"x0q{qtr}")
                        nc.sync.dma_start(out=x4[:],
                                          in_=xt_r[:, 4*qtr:4*qtr+4, 0:512])
                        x0q.append(x4)
                    wk_sb = wpool.tile([128, 16, 512], BF16, tag="wk")
                    nc.sync.dma_start(out=wk_sb[:], in_=wkt_r[:, :, p_*512:(p_+1)*512])
                    wv_sb = wpool.tile([128, 16, 512], BF16, tag="wv")
                    nc.sync.dma_start(out=wv_sb[:], in_=wvt_r[:, :, p_*512:(p_+1)*512])
                    if p_ == 0:
                        # biases behind the critical first weight/x loads (the
                        # bv partition-broadcast descriptor-gen is slow)
                        nc.sync.dma_start(out=bq_sb[:], in_=bqt[:])
                        nc.sync.dma_start(out=bk_sb[:], in_=bkt[:])
                        for pb in range(2):
                            nc.sync.dma_start(
                                out=bv_sb[:, pb, :],
                                in_=bvv[pb*512:(pb+1)*512]
                                    .partition_broadcast(128))
                        nc.vector.memset(ones_bf[:], 1.0)
                    for sc in range(NSC):
                        if sc == 0:
                            xs = x0q
                        else:
                            xs = []
                            for qtr in range(4):
                                x4 = xpool.tile([128, 4, 512], BF16, tag="xin")
                                nc.sync.dma_start(
                                    out=x4[:],
                                    in_=xt_r[:, 4*qtr:4*qtr+4,
                                             sc*512:(sc+1)*512])
                                xs.append(x4)
                        for hh in range(4):
                            h = p_ * 4 + hh
                            qps = qkp.tile([128, 512], F32, tag="qk")
                            for dc in range(16):
                                nc.tensor.matmul(
                                    qps[:],
                                    wqq[dc//4][:, dc % 4, hh*128:(hh+1)*128],
                                    xs[dc//4][:, dc % 4, :],
                                    start=(dc == 0), stop=(dc == 15))
                            q_sb = spool.tile([128, 512], BF16, tag="qko")
                            nc.scalar.activation(q_sb[:], qps[:], AF.Identity,
                                                 bias=bq_sb[:, h:h+1], scale=1.0)
                            nc.sync.dma_start(out=qd[h][:, sc*512:(sc+1)*512], in_=q_sb[:])

                            kps = qkp.tile([128, 512], F32, tag="qk")
                            for dc in range(16):
                                nc.tensor.matmul(
                                    kps[:], wk_sb[:, dc, hh*128:(hh+1)*128],
                                    xs[dc//4][:, dc % 4, :],
                                    start=(dc == 0), stop=(dc == 15))
                            k_sb = spool.tile([128, 512], BF16, tag="qko")
                            nc.scalar.activation(k_sb[:], kps[:], AF.Identity,
                                                 bias=bk_sb[:, h:h+1], scale=1.0)
                            nc.sync.dma_start(out=kd[h][:, sc*512:(sc+1)*512], in_=k_sb[:])
                        # V for this pass: [k, dv] chunks (4 heads' dv)
                        for kc in range(4):
                            vp = vps.tile([128, 512], F32, tag="v")
                            for dc in range(16):
                                nc.tensor.matmul(
                                    vp[:],
                                    xs[dc//4][:, dc % 4, kc*128:(kc+1)*128],
                                    wv_sb[:, dc, :], start=(dc == 0), stop=(dc == 15))
                            v_sb = spool.tile([128, 512], BF16, tag="vo")
                            nc.vector.tensor_add(v_sb[:], vp[:], bv_sb[:, p_, :])
                            for hh in range(4):
                                h = p_ * 4 + hh
                                nc.sync.dma_start(
                                    out=vd[h][:, sc*4+kc, :],
                                    in_=v_sb[:, hh*128:(hh+1)*128])

            # ------------- Stage 2: attention, qc-outer -------------
            with tc.tile_pool(name="s2kv", bufs=1) as kvpool, \
                 tc.tile_pool(name="s2wo", bufs=1) as wopool, \
                 tc.tile_pool(name="s2q3", bufs=12) as q3pool, \
                 tc.tile_pool(name="s2es", bufs=2) as espool, \
                 tc.tile_pool(name="s2sm", bufs=3) as smpool, \
                 tc.tile_pool(name="s3o", bufs=2) as opool, \
                 tc.tile_pool(name="psp", bufs=2, space="PSUM") as psp, \
                 tc.tile_pool(name="pcd", bufs=2, space="PSUM") as pcd, \
                 tc.tile_pool(name="pop", bufs=2, space="PSUM") as pop:
                # heads 4-7's K/V/Q spills land only at stage-1 end, so
                # run heads 0-3's qc=0/1 units first to cover the latency
                units = ([(0, h) for h in range(4)]
                         + [(1, h) for h in range(4)]
                         + [(0, h) for h in range(4, HPC)]
                         + [(1, h) for h in range(4, HPC)]
                         + [(qc, h) for qc in (2, 3) for h in range(HPC)])
                NU = len(units)
                # normalized context, SBUF-resident through the out-proj
                ctxA = kvpool.tile([128, HPC, NSC, 512], BF16, tag="ctxA",
                                   name="ctxA")
                # pass-1 heads' K/V resident too (bf16: 8 MiB all-head total),
                # plus Wo and the remaining first-qc Q chunks
                for h in range(4, HPC):
                    k2h[h] = kvpool.tile([128, seq], BF16, tag=f"k2_{h}", name=f"k2_{h}")
                    v2h[h] = kvpool.tile([128, NKC, 128], BF16, tag=f"v2_{h}", name=f"v2_{h}")
                    for sc in range(NSC):
                        nc.sync.dma_start(
                            out=k2h[h][:, sc*512:(sc+1)*512],
                            in_=kd[h][:, sc*512:(sc+1)*512])
                        nc.sync.dma_start(
                            out=v2h[h][:, sc*4:(sc+1)*4, :],
                            in_=vd[h][:, sc*4:(sc+1)*4, :])
                wos = []
                for ec in range(4):
                    wo_sb = wopool.tile([128, HPC, 512], BF16, tag=f"wo_{ec}")
                    nc.sync.dma_start(out=wo_sb[:],
                                      in_=wot_r[:, :, ec*512:(ec+1)*512])
                    wos.append(wo_sb)
                for qc, h in units[4:8]:
                    q3 = q3pool.tile([128, 512], BF16, tag="q3")
                    nc.sync.dma_start(out=q3[:],
                                      in_=qd[h][:, qc*512:(qc+1)*512])
                    q3sb[(qc, h)] = q3

                done_pos = {qc: max(i for i, u in enumerate(units)
                                    if u[0] == qc) for qc in range(NSC)}

                # out-projection sub-blocks: one per iteration, starting two
                # iterations after the last ctxn of that qc (PE interleave)
                mm_at = {}
                pending_op = []
                for qc in sorted(done_pos, key=lambda q: done_pos[q]):
                    for ss in range(4):
                        pending_op.append((done_pos[qc] + 2, qc, ss))
                slot = 0
                for ready, qc, ss in pending_op:
                    slot = max(slot, ready)
                    if slot < NU + 2:
                        mm_at.setdefault(slot, []).append((qc, ss))
                        slot += 1
                    # later blocks flush in the tail

                state = {}     # (qc,h) -> (es, acc, ctxp, rcp)

                def emit_outproj_mm(qc, ss):
                    for ec in range(4):
                        op = pop.tile([128, 512], F32, tag="op")
                        for h in range(HPC):
                            nc.tensor.matmul(
                                op[:],
                                ctxA[:, h, qc, ss*128:(ss+1)*128],
                                wos[ec][:, h, :],
                                start=(h == 0), stop=(h == HPC - 1))
                        o_sb = opool.tile([128, 512], F32, tag="o")
                        nc.scalar.activation(o_sb[:], op[:], AF.Copy)
                        nc.sync.dma_start(
                            out=out[qc*512+ss*128:qc*512+(ss+1)*128,
                                    ec*512:(ec+1)*512],
                            in_=o_sb[:])

                for it in range(NU + 2):
                    # --- interleaved out-projection sub-blocks ---
                    for qs in mm_at.get(it, ()):
                        emit_outproj_mm(*qs)
                    # --- produce unit it: scores + exp + mask + den-acc ---
                    if it < NU:
                        qc, h = units[it]
                        nk = 4 * qc + 4
                        if it + 8 < NU:    # prefetch q3 eight units ahead
                            pqc, ph = units[it + 8]
                            if (pqc, ph) not in q3sb:
                                q3n = q3pool.tile([128, 512], BF16, tag="q3")
                                nc.sync.dma_start(
                                    out=q3n[:],
                                    in_=qd[ph][:, pqc*512:(pqc+1)*512])
                                q3sb[(pqc, ph)] = q3n
                        q3 = q3sb.pop((qc, h))
                        k2 = k2h[h]
                        es = espool.tile([128, NKC, 512], BF16, tag="es")
                        acc = smpool.tile([128, 512], BF16, tag="acc")
                        for pi in range(nk // 2):
                            k0, k1 = 2 * pi, 2 * pi + 1
                            j0, j1 = k0 - 4 * qc, k1 - 4 * qc
                            lo0 = 128 * j0 if j0 > 0 else 0
                            lo1 = 128 * j1 if j1 > 0 else 0
                            sp = psp.tile([128, 2, 512], F32, tag="sp")
                            nc.tensor.matmul(
                                sp[:, 0, lo0:], k2[:, k0*128:(k0+1)*128],
                                q3[:, lo0:], start=True, stop=True)
                            nc.tensor.matmul(
                                sp[:, 1, lo1:], k2[:, k1*128:(k1+1)*128],
                                q3[:, lo1:], start=True, stop=True)
                            nc.scalar.activation(
                                es[:, k0:k1+1, lo0:], sp[:, :, lo0:],
                                AF.Exp, bias=0.0, scale=SCALE)
                            for kk, jj in ((k0, j0), (k1, j1)):
                                if jj >= 0:   # diagonal: causal mask (fill 0)
                                    nc.gpsimd.affine_select(
                                        out=es[:, kk, :], in_=es[:, kk, :],
                                        compare_op=mybir.AluOpType.is_ge,
                                        fill=0.0, base=-128 * jj,
                                        pattern=[[1, 512]],
                                        channel_multiplier=-1)
                            # denominator accumulation on DVE (diagonal
                            # chunks enter after their affine_select)
                            for kk, jj in ((k0, j0), (k1, j1)):
                                if kk == 0:
                                    nc.vector.tensor_copy(acc[:], es[:, 0, :])
                                else:
                                    nc.vector.tensor_add(
                                        acc[:], acc[:], es[:, kk, :])
                        state[(qc, h)] = (es, acc, None, None)
                    # --- consume unit it-1: ctx matmuls (PE) ---
                    if 1 <= it <= NU:
                        qc, h = units[it - 1]
                        nk = 4 * qc + 4
                        es, acc, _, _ = state[(qc, h)]
                        v2 = v2h[h]
                        ctxp = pcd.tile([128, 512], F32, tag="cd")
                        for kc in range(nk):
                            j = kc - 4 * qc
                            lo = 128 * j if j > 0 else 0
                            nc.tensor.matmul(
                                ctxp[:, lo:], v2[:, kc, :], es[:, kc, lo:],
                                start=(kc == 0), stop=(kc == nk - 1))
                        # den[p, q] = sum_k acc[k, q] for every p: all-ones
                        # stationary matmul reduces over partitions AND
                        # broadcasts in one 512-cycle op; placed after the
                        # ctx matmuls so the PE never waits on the DVE chain
                        den_ps = psp.tile([128, 2, 512], F32, tag="sp")
                        nc.tensor.matmul(den_ps[:, 0, :], ones_bf[:], acc[:],
                                         start=True, stop=True)
                        # 1/den as Exp(-Ln(den)): the natural_log_exp table
                        # covers Ln/Exp/Identity/Copy -> no table reloads
                        lnd = smpool.tile([128, 512], F32, tag="lnd")
                        nc.scalar.activation(lnd[:], den_ps[:, 0, :], AF.Ln)
                        rcp = smpool.tile([128, 512], F32, tag="rcp")
                        nc.scalar.activation(rcp[:], lnd[:], AF.Exp,
                                             scale=-1.0)
                        # normalize straight out of PSUM into resident SBUF
                        nc.vector.tensor_mul(ctxA[:, h, qc, :], ctxp[:],
                                             rcp[:])
                        del state[(qc, h)]

                # tail: flush remaining out-projection sub-blocks
                emitted = {qs for lst in mm_at.values() for qs in lst}
                for _, qc, ss in pending_op:
                    if (qc, ss) not in emitted:
                        emit_outproj_mm(qc, ss)
    split_excess_waits(nc)
    return nc


_NC_CACHE = {}


def _get_nc(seq):
    if seq not in _NC_CACHE:
        _NC_CACHE[seq] = build_nc(seq)
    return _NC_CACHE[seq]


def make_in_maps(x, Wq, bq, Wk, bk, Wv, bv, Wo, bo, seq=S, nb=B):
    import ml_dtypes
    f32 = np.float32
    bf16 = ml_dtypes.bfloat16
    in_maps = []
    for c in range(NCORES):
        b = c // 2
        half = c % 2
        sl = slice(half * MLOC, (half + 1) * MLOC)
        in_maps.append({
            "xt": np.ascontiguousarray(x[b].T.astype(bf16)),
            "wqt": np.ascontiguousarray(Wq[sl, :].T.astype(bf16)),
            "wkt": np.ascontiguousarray(Wk[sl, :].T.astype(bf16)),
            "wvt": np.ascontiguousarray(Wv[sl, :].T.astype(bf16)),
            "wot": np.ascontiguousarray(Wo[:, sl].T.astype(bf16)),
            "bqt": np.ascontiguousarray(bq[sl].reshape(HPC, DK).T, dtype=f32),
            "bkt": np.ascontiguousarray(bk[sl].reshape(HPC, DK).T, dtype=f32),
            "bvv": np.ascontiguousarray(bv[sl], dtype=f32),
        })
    return in_maps


def run(inputs, trace=False, trace_kwargs=None):
    x = np.asarray(inputs["x"], dtype=np.float32)
    nb, seq, d = x.shape
    nc = _get_nc(seq)
    in_maps = make_in_maps(
        x, np.asarray(inputs["Wq"]), np.asarray(inputs["bq"]),
        np.asarray(inputs["Wk"]), np.asarray(inputs["bk"]),
        np.asarray(inputs["Wv"]), np.asarray(inputs["bv"]),
        np.asarray(inputs["Wo"]), np.asarray(inputs["bo"]), seq=seq, nb=nb)
    res = run_bass_kernel_spmd(nc, in_maps, list(range(NCORES)), trace=trace,
                               **(trace_kwargs or {}))
    bo = np.asarray(inputs["bo"], dtype=np.float32)
    out = np.empty((nb, seq, d), dtype=np.float32)
    for b in range(nb):
        out[b] = res.results[2*b]["out"] + res.results[2*b+1]["out"] + bo
    return out, res


def kernel(**inputs):
    out, _ = run(inputs, trace=False)
    return out
